# revision 123
# baseline (speedup 1.0000x reference)
"""Trainium2 Bass kernel for DifferentiableBiquadChain.

Math: per (batch, frame) lane, the 16-biquad cascade is an LTI filter applied
from zero state to a 2048-sample frame.  We decompose the transfer function by
partial fractions over the 16 stage pole-pairs (handled uniformly in the
algebra R[w]/(w^2 - disc) so complex and real pole pairs share one code path).
The frame is processed in 16 blocks of 128 samples:
  y_j[r] = sum_c h[r-c] x_j[c]                (within-block, PE matmul A1)
         + sum_slots beta_slot[j] S_slot[r+1] (carry of all previous blocks,
                                               PE matmul B)
where the 32 "slots" per lane are the (A,B) components of the 16 pole pairs,
S are slot power sequences, and beta comes from a 16-step block-state scan
(computed on-device from the Wend matmuls + vector-engine scan).

The device executes the whole audio data path AND most table generation:
 - audio ships as raw bf16 [BPC, N]; per-lane [c, j] layout comes from paired
   PE transposes (quads of frames, bf16)
 - the slot power tables S_q[0..128] are generated on device by a 7-step
   doubling recursion (DVE + partner-row swap DMAs) from per-lane seeds
 - pt (the Wend stationary layout) comes from per-lane PE transposes of the
   power table; the impulse response h[127-r] falls out of the same slice via
   a [32,1]x[32,128] matmul per lane, with Dt folded in f32 before narrowing
 - within-block convolution (PE, bf16), block-end Wend matmuls (PE, f32),
   cross-block state scan (DVE), carry matmuls (PE, f32), y evict in bf16
Only the per-lane biquad coefficients / residues (cancellation-sensitive,
float64) stay on the host: ~170 KB of [64,128] seed tables per call.

Precision notes (measured): pts/zPr/beta/carry must stay f32 — the Wend
projections feed the huge-residue beta cancellation and bf16 there blows
resonant lanes up 20x.  X/hd/y in bf16 are safe (worst-lane ~2.6e-2,
aggregate ~2.6e-3 vs the 2e-2 gate).  Dt must be added to h[0] before any
bf16 narrowing or attenuated lanes lose their direct-path cancellation.

Wall-clock structure per call (axon-tunneled PJRT, ~60-80 MB/s): bf16 audio
upload (8.4 MB, overlapped with host table prep), tiny table upload, ~0.7 ms
device exec, bf16 y download (8.4 MB).  The jitted shard_map callable and the
donated zero output buffers are cached/prefetched across calls.
"""

import math
import sys

import numpy as np

sys.path.insert(0, "/opt/trn_rl_repo")

SR = 96000.0
FRAME = 2048
NB = 16
L = 128
NJ = 16
B_FULL, F = 16, 128
N = F * FRAME
N_CORES = 8
BPC = B_FULL // N_CORES          # batches per core = 2
NL = BPC * F                     # lanes per core = 256
GAIN_RANGE = (-24.0, 24.0)
BROADBAND = (-60.0, 0.0)
Q_RANGE = (0.5, 16.0)
HPF_R = (20.0, 500.0)
LPF_R = (5000.0, 20000.0)
SHELF_R = (50.0, 16000.0)
PEAK_R = (100.0, 15000.0)
DMIN = 1e-8

# ---------------------------------------------------------------- host setup


def _denorm_freq(n, r):
    lo, hi = math.log(r[0]), math.log(r[1])
    return np.exp(lo + n * (hi - lo))


def _coeffs(params):
    B = params.shape[0]
    p = params.astype(np.float64)
    nl = B * F
    b0 = np.zeros((NB, nl)); b1 = np.zeros((NB, nl)); b2 = np.zeros((NB, nl))
    a1 = np.zeros((NB, nl)); a2 = np.zeros((NB, nl))
    for i in range(NB):
        fn = p[:, 3 * i, :].reshape(nl)
        gn = p[:, 3 * i + 1, :].reshape(nl)
        qn = p[:, 3 * i + 2, :].reshape(nl)
        Q = np.exp(math.log(Q_RANGE[0]) + qn * (math.log(Q_RANGE[1]) - math.log(Q_RANGE[0])))
        g = GAIN_RANGE[0] + gn * (GAIN_RANGE[1] - GAIN_RANGE[0])
        A = 10.0 ** (g / 40.0)
        if i == 0:
            fc, typ = _denorm_freq(fn, HPF_R), "hp"
        elif i == NB - 1:
            fc, typ = _denorm_freq(fn, LPF_R), "lp"
        elif i == 1:
            fc, typ = _denorm_freq(fn, SHELF_R), "ls"
        elif i == NB - 2:
            fc, typ = _denorm_freq(fn, SHELF_R), "hs"
        else:
            fc, typ = _denorm_freq(fn, PEAK_R), "pk"
        w0 = 2 * math.pi * fc / SR
        al = np.sin(w0) / (2 * Q)
        c = np.cos(w0)
        sA = np.sqrt(A)
        if typ == "hp":
            B0, B1, B2, A0, A1_, A2_ = (1 + c) / 2, -(1 + c), (1 + c) / 2, 1 + al, -2 * c, 1 - al
        elif typ == "lp":
            B0, B1, B2, A0, A1_, A2_ = (1 - c) / 2, 1 - c, (1 - c) / 2, 1 + al, -2 * c, 1 - al
        elif typ == "pk":
            B0, B1, B2, A0, A1_, A2_ = 1 + al * A, -2 * c, 1 - al * A, 1 + al / A, -2 * c, 1 - al / A
        elif typ == "ls":
            B0 = A * (A + 1 - (A - 1) * c + 2 * sA * al); B1 = 2 * A * (A - 1 - (A + 1) * c)
            B2 = A * (A + 1 - (A - 1) * c - 2 * sA * al)
            A0 = A + 1 + (A - 1) * c + 2 * sA * al; A1_ = -2 * (A - 1 + (A + 1) * c)
            A2_ = A + 1 + (A - 1) * c - 2 * sA * al
        else:
            B0 = A * (A + 1 + (A - 1) * c + 2 * sA * al); B1 = -2 * A * (A - 1 + (A + 1) * c)
            B2 = A * (A + 1 + (A - 1) * c - 2 * sA * al)
            A0 = A + 1 - (A - 1) * c + 2 * sA * al; A1_ = 2 * (A - 1 - (A + 1) * c)
            A2_ = A + 1 - (A - 1) * c - 2 * sA * al
        b0[i] = B0 / A0; b1[i] = B1 / A0; b2[i] = B2 / A0
        a1[i] = A1_ / A0; a2[i] = A2_ / A0
    n48 = p[:, 48, :].reshape(nl); n49 = p[:, 49, :].reshape(nl)
    gio = 10.0 ** (((BROADBAND[0] + n48 * 60.0) + (BROADBAND[0] + n49 * 60.0)) / 20.0)
    return b0, b1, b2, a1, a2, gio


def _pair_setup(b0, b1, b2, a1, a2, gio):
    disc = a1 * a1 / 4 - a2
    disc = np.where(np.abs(disc) > DMIN, disc, DMIN)
    s = np.sqrt(np.abs(disc))
    eps = np.sign(disc)
    h0 = -a1 / 2
    di = disc[:, None, :]
    wiA = (h0 / a2)[:, None, :]; wiB = (-1.0 / a2)[:, None, :]
    w2A = wiA * wiA + di * wiB * wiB
    w2B = 2 * wiA * wiB
    BA = b0[None] + b1[None] * wiA + b2[None] * w2A
    BB = b1[None] * wiB + b2[None] * w2B
    AA = 1.0 + a1[None] * wiA + a2[None] * w2A
    AB = a1[None] * wiB + a2[None] * w2B
    eye = np.eye(NB, dtype=bool)[:, :, None]
    AA = np.where(eye, 1.0, AA); AB = np.where(eye, 0.0, AB)
    n = AA * AA - di * AB * AB
    RA = (BA * AA - di * BB * AB) / n
    RB = (BB * AA - BA * AB) / n
    PA = RA[:, 0, :]; PB = RB[:, 0, :]
    for j in range(1, NB):
        PA, PB = (PA * RA[:, j] + disc * PB * RB[:, j], PA * RB[:, j] + PB * RA[:, j])
    dA = (a2 - h0 * h0 - disc) / a2; dB = 2 * h0 / a2
    nn = dA * dA - disc * dB * dB
    aA = (PA * dA - disc * PB * dB) / nn
    aB = (PB * dA - PA * dB) / nn
    cA = 2 * aA * gio
    cB = 2 * disc * aB / s * gio
    Dt = np.prod(b2, axis=0) / np.prod(a2, axis=0) * gio
    return h0, s, eps, cA, cB, Dt


def _slot_powers(h0, s, eps, n_max):
    sh = h0.shape
    SA = np.zeros(sh + (n_max + 1,)); SB = np.zeros_like(SA)
    SA[..., 0] = 1.0
    SA[..., 1] = h0; SB[..., 1] = s
    m = 1
    while m < n_max:
        t = min(m, n_max - m)
        mulA = SA[..., m:m + 1]; mulB = SB[..., m:m + 1]
        mulBe = eps[..., None] * mulB
        newA = SA[..., 1:1 + t] * mulA + SB[..., 1:1 + t] * mulBe
        newB = SA[..., 1:1 + t] * mulB + SB[..., 1:1 + t] * mulA
        SA[..., m + 1:m + 1 + t] = newA; SB[..., m + 1:m + 1 + t] = newB
        m *= 2
    return SA, SB


def host_tables_all(audio, params):
    """Global (all-8-core concatenated) input arrays, vectorized across cores.

    Per-core table layouts are documented below; each global array stacks the
    8 per-core arrays along axis 0 in core order, matching shard_map's
    in_specs=P('core') split.  The audio itself ships separately (bf16 cast
    of the raw [16, N] array; the device does the lane transpose).
    """
    b0, b1, b2, a1, a2, gio = _coeffs(params)        # nl = 16*F lanes
    h0, s, eps, cA, cB, Dt = _pair_setup(b0, b1, b2, a1, a2, gio)

    def tocore(x):   # (16, nl) -> [core, lgp, row16, ll]
        x6 = x.reshape(16, N_CORES, BPC, 2, 64)      # row, core, b, lgp, f6
        return x6.transpose(1, 3, 0, 2, 4).reshape(N_CORES, 2, 16, 128)
    def pack(ahalf, bhalf):                          # -> [8*64, 128]
        z = np.empty((N_CORES, 2, 2, 16, 128), np.float64)  # core, lgp, half, row, ll
        z[:, :, 0] = tocore(ahalf); z[:, :, 1] = tocore(bhalf)
        return z.reshape(N_CORES * 64, 128).astype(np.float32)
    Dtt = np.ascontiguousarray(Dt.reshape(N_CORES, NL)).astype(np.float32)
    return {
        "S1t": pack(h0, s),
        "epst": pack(eps, np.ones_like(eps)),
        "cAt": pack(cA, eps * cA),
        "cBt": pack(cB, cB),
        "cht": pack(cA, cB),
        "Dtt": Dtt,
    }


# Per-core table layouts (rows are SBUF partitions):
#   hz     [128, 2*264]  f-partition rows; h[m] at col b*264 + (136 - m), zeros
#                        outside [9,136] so the sliding hd gather reads h[127-c-rp]
#   aA_t   [64, 128]     rows 32*lgp + 16*half + i, cols ll = b*64+f6:
#                        A-mult of the z-step for that slot row
#   aBe_t  [64, 128]     swapped-operand multiplier for the z-step
#   cA_t   [64, 128]     beta combine: beta_row_q = cA_t*z_q + cB_t*zswap_q
#   cB_t   [64, 128]
#   S1_t   [64, 128]     S_q[1] seed for the on-device power doubling (h0 | s)
#   eps_t  [64, 128]     eps on A rows, 1.0 on B rows (doubling cross-term sign)
# z-step (per slot pair, uniform rows):  z' = aA_t*z + aBe_t*zswap + w
#   rows q<16 (A): zA' = sA128*zA + eps*sB128*zB   -> aA_t=sA128, aBe_t=eps*sB128
#   rows q>=16(B): zB' = sA128*zB + sB128*zA       -> aA_t=sA128, aBe_t=sB128
# beta:  bA = cA*zA + cB*zB       -> rows A: cA_t=cA, cB_t=cB
#        bB = cB*zA + eps*cA*zB   -> rows B: beta_B = cA_t*zB + cB_t*zA with
#                                    cA_t=eps*cA, cB_t=cB.

# ---------------------------------------------------------------- device code

_prog_cache = {}


def _build_program(split_waits=True):
    import concourse.bass as bass
    import concourse.tile as tile
    import concourse.mybir as mb
    import bass_rust
    from concourse.masks import make_identity

    f32 = mb.dt.float32
    bf16 = mb.dt.bfloat16
    Alu = mb.AluOpType
    nc = bass.Bass("TRN2", target_bir_lowering=False, debug=False)

    xT = nc.dram_tensor("xT", [BPC, N], bf16, kind="ExternalInput").ap()
    S1_d = nc.dram_tensor("S1t", [64, 128], f32, kind="ExternalInput").ap()
    eps_d = nc.dram_tensor("epst", [64, 128], f32, kind="ExternalInput").ap()
    cA_d = nc.dram_tensor("cAt", [64, 128], f32, kind="ExternalInput").ap()
    cB_d = nc.dram_tensor("cBt", [64, 128], f32, kind="ExternalInput").ap()
    ch_d = nc.dram_tensor("cht", [64, 128], f32, kind="ExternalInput").ap()
    Dt_d = nc.dram_tensor("Dtt", [1, NL], f32, kind="ExternalInput").ap()
    y_d = nc.dram_tensor("y", [BPC, N], bf16, kind="ExternalOutput").ap()

    with tile.TileContext(nc) as tc:
        with tc.tile_pool(name="big", bufs=1) as big, \
             tc.tile_pool(name="zsc", bufs=1) as zsc, \
             tc.tile_pool(name="dbl", bufs=1) as dbl, \
             tc.tile_pool(name="hd", bufs=8) as hdp, \
             tc.tile_pool(name="hb", bufs=1) as hbp, \
             tc.tile_pool(name="yb", bufs=2) as ybp, \
             tc.tile_pool(name="psA", bufs=4, space="PSUM") as psA, \
             tc.tile_pool(name="psT", bufs=3, space="PSUM") as psT, \
             tc.tile_pool(name="psQ", bufs=1, space="PSUM") as psQ:

            # FR: raw audio frames; partition p = (f%8)*16 + j, col b*2048 +
            # (f>>3)*128 + c.  audio flat idx = b*N + f8*16384 + p*128 + c.
            FR = big.tile([128, BPC * 16 * 128], bf16, name="FR")
            X = big.tile([128, NL * 16], bf16, name="X")
            hzs = big.tile([128, BPC * 264], bf16, name="hzs")
            # zPr: S_q power table, REVERSED free index k = 128 - m.
            # rows 32*lgp + q; col ll*129 + k holds S_q[128-k] for lane lgp,ll.
            zPr = big.tile([64, 128 * 129], f32, name="zPr")
            # pts stays f32: the Wend projections feed the huge-residue beta
            # cancellation; bf16 here blows resonant lanes up (tested: 20x)
            pts = big.tile([128, NL * 32], f32, name="pts")
            Xf = big.tile([128, NL * 16], f32, name="Xf")
            S1s = big.tile([64, 128], f32, name="S1s")
            epss = big.tile([64, 128], f32, name="epss")
            aAs = big.tile([64, 128], f32, name="aAs")
            aBes = big.tile([64, 128], f32, name="aBes")
            cAs = big.tile([64, 128], f32, name="cAs")
            cBs = big.tile([64, 128], f32, name="cBs")
            chs = big.tile([64, 128], f32, name="chs")
            Dts = big.tile([1, NL], f32, name="Dts")
            id32 = big.tile([64, 32], f32, name="id32")
            id32b = big.tile([128, 64], bf16, name="id32b")
            for b in range(BPC):
                src = xT[b:b + 1, :].copy()
                src.ap = bass_rust.VecI64Pair([[128, 128], [16384, 16], [1, 128]])
                src.offset = b * N
                nc.sync.dma_start(FR[:, b * 2048:(b + 1) * 2048], src)
            nc.sync.dma_start(S1s[:], S1_d[:, :])
            nc.sync.dma_start(epss[:], eps_d[:, :])
            nc.sync.dma_start(cAs[:], cA_d[:, :])
            nc.sync.dma_start(cBs[:], cB_d[:, :])
            nc.sync.dma_start(chs[:], ch_d[:, :])
            nc.sync.dma_start(Dts[:], Dt_d[:, :])
            make_identity(nc, id32[0:32, :])
            make_identity(nc, id32[32:64, :])
            make_identity(nc, id32b[0:64, :])
            make_identity(nc, id32b[64:128, :])
            nc.vector.memset(hzs[:], 0.0)            # h written per-lane below

            # ---- X (f32, [c, lane*16+j]) from FR via quad PE transposes:
            # frames (f..f+3) share a col block; [64,128] -> [128,64] covers
            # four consecutive lanes (matmul bases must be 0/32/64).
            for b in range(BPC):
                for f in range(0, F, 4):
                    base = (f % 8) * 16
                    col = b * 2048 + (f >> 3) * 128
                    lane = b * 128 + f
                    pq = psQ.tile([128, 64], bf16, name="pq")
                    nc.tensor.matmul(pq[:], FR[base:base + 64, col:col + 128],
                                     id32b[base:base + 64, :], is_transpose=True,
                                     start=True, stop=True, skip_group_check=True)
                    # scalar engine widening from psum is proven on HW; keep
                    # the same-dtype bf16 evict on DVE to be safe
                    nc.vector.tensor_copy(X[:, lane * 16:lane * 16 + 64], pq[:])
                    nc.scalar.copy(Xf[:, lane * 16:lane * 16 + 64], pq[:])

            # ---- on-device slot-power doubling: build zPr from S1/eps.
            # k-MAJOR layout: zPr[p, k*128 + ll] holds S_q[128-k] for lane ll;
            # every block the doubling touches is a CONTIGUOUS column range, so
            # the partner-row swap DMAs are few fat descriptors (the previous
            # lane-major layout cost ~200us in element-granular swap DMAs).
            # Host reference (_slot_powers): newA = SA[1..t]*SA[m] + SB[1..t]*(eps*SB[m])
            #                                newB = SA[1..t]*SB[m] + SB[1..t]*SA[m]
            # Row space: A rows [0:16]/[32:48], B rows [16:32]/[48:64] per lgp.
            def zcol(p0, np_, k):
                return zPr[p0:p0 + np_, k * 128:(k + 1) * 128]

            swp = dbl.tile([64, 32 * 128], f32, name="swp")
            tmp = dbl.tile([64, 32 * 128], f32, name="tmp")
            M1 = dbl.tile([64, 128], f32, name="M1")
            M2 = dbl.tile([64, 128], f32, name="M2")
            # init: S[0]=(1,0) at k=128, S[1]=(h0,s) at k=127
            # (compute engines only address partition starts 0/32)
            nc.vector.memset(zcol(0, 64, 128), 0.0)
            nc.vector.memset(zcol(0, 16, 128), 1.0)
            nc.vector.memset(zcol(32, 16, 128), 1.0)
            nc.scalar.copy(zcol(0, 64, 127), S1s[:])
            # (m, ja, ts): dst S[m+ja+1 .. m+ja+ts] = S[ja+1..ja+ts] (x) S[m];
            # the m=64 step splits its j-range to bound swp/tmp at 16 KB/part
            steps = [(1, 0, 1), (2, 0, 2), (4, 0, 4), (8, 0, 8), (16, 0, 16),
                     (32, 0, 32), (64, 0, 32), (64, 32, 32)]
            m_prev = None
            for (m, ja, ts) in steps:
                ks = 128 - m
                if m != m_prev:
                    # M1 = SA[m] on all rows; M2 = (eps|1) * SB[m] on all rows
                    nc.scalar.copy(M1[0:16, :], zcol(0, 16, ks))
                    nc.scalar.copy(M1[32:48, :], zcol(32, 16, ks))
                    nc.sync.dma_start(M1[16:32, :], zcol(0, 16, ks))
                    nc.sync.dma_start(M1[48:64, :], zcol(32, 16, ks))
                    nc.sync.dma_start(M2[16:32, :], zcol(16, 16, ks))
                    nc.sync.dma_start(M2[48:64, :], zcol(48, 16, ks))
                    nc.sync.dma_start(M2[0:16, :], zcol(16, 16, ks))
                    nc.sync.dma_start(M2[32:48, :], zcol(48, 16, ks))
                    nc.gpsimd.tensor_tensor(M2[:], M2[:], epss[:], op=Alu.mult)
                    m_prev = m
                # contiguous col ranges: src j in (ja, ja+ts], dst j+m
                slo = (128 - ja - ts) * 128
                src = zPr[0:64, slo:slo + ts * 128]
                dst = zPr[0:64, slo - m * 128:slo - m * 128 + ts * 128]
                for qi, (pd, ps) in enumerate(((0, 16), (16, 0), (32, 48), (48, 32))):
                    eng = nc.sync if qi % 2 == 0 else nc.scalar
                    eng.dma_start(swp[pd:pd + 16, 0:ts * 128],
                                  zPr[ps:ps + 16, slo:slo + ts * 128])
                m1b = M1[:].unsqueeze(1).broadcast_to([64, ts, 128])
                s3 = src.rearrange("p (t l) -> p t l", t=ts)
                d3 = dst.rearrange("p (t l) -> p t l", t=ts)
                # source-term product has no swap dependency — full width on DVE
                nc.vector.tensor_tensor(d3, s3, m1b, op=Alu.mult)
                # swap-dependent mult+add split by partition half so each half
                # chains off its own swap round instead of waiting for all four
                for ph in (0, 32):
                    m2b = M2[ph:ph + 32, :].unsqueeze(1).broadcast_to([32, ts, 128])
                    w3 = swp[ph:ph + 32, 0:ts * 128].rearrange("p (t l) -> p t l", t=ts)
                    t3 = tmp[ph:ph + 32, 0:ts * 128].rearrange("p (t l) -> p t l", t=ts)
                    nc.gpsimd.tensor_tensor(t3, w3, m2b, op=Alu.mult)
                    nc.vector.tensor_tensor(dst[ph:ph + 32, :], dst[ph:ph + 32, :],
                                            tmp[ph:ph + 32, 0:ts * 128], op=Alu.add)

            # ---- aAs/aBes (z-scan multipliers = S[128] scalars) from zPr k=0
            nc.scalar.copy(aAs[0:16, :], zcol(0, 16, 0))
            nc.scalar.copy(aAs[32:48, :], zcol(32, 16, 0))
            nc.sync.dma_start(aAs[16:32, :], zcol(0, 16, 0))
            nc.sync.dma_start(aAs[48:64, :], zcol(32, 16, 0))
            nc.sync.dma_start(aBes[16:32, :], zcol(16, 16, 0))
            nc.sync.dma_start(aBes[48:64, :], zcol(48, 16, 0))
            nc.sync.dma_start(aBes[0:16, :], zcol(16, 16, 0))
            nc.sync.dma_start(aBes[32:48, :], zcol(48, 16, 0))
            nc.vector.tensor_tensor(aBes[:], aBes[:], epss[:], op=Alu.mult)

            # ---- pts + h from zPr: per lane, PE-transpose [32 q, 128] of
            # S[127-c] (zPr col (1+c)*128 + ll, c ascending -> stride 128),
            # and the impulse response h[127-r] = sum_q ch_q S_q[127-r] via a
            # [32,1]x[32,128] matmul on the SAME slice -> pre-reversed for hz.
            # Wend groups are interleaved: group llo fires as soon as its 8
            # lanes' pts columns exist, so wendb completes with the loop and
            # the z-scan starts ~25us earlier.
            wendb = zsc.tile([64, 16 * 128], f32, name="wendb")   # rows (lgp,q), cols j*128+ll

            def emit_wend(llo):
                pw = psA.tile([64, 256], f32, name="pa")   # only cols 0:64 used
                for lli in range(4):
                    ll = llo * 4 + lli
                    for lgp in range(2):
                        lane = (ll // 64) * 128 + lgp * 64 + (ll % 64)
                        nc.tensor.matmul(
                            pw[32 * lgp:32 * lgp + 32, lli * 16:lli * 16 + 16],
                            pts[:, lane * 32:lane * 32 + 32],
                            Xf[:, lane * 16:lane * 16 + 16],
                            start=True, stop=True, skip_group_check=True)
                # evict: pw rows (lgp,q), cols (lli,j) -> wendb cols j*128 + llo*4+lli
                src = pw[:, 0:64].rearrange("p (l j) -> p l j", l=4)
                dst = wendb[:].copy()
                dst.ap = bass_rust.VecI64Pair([[dst.ap[0][0], 64], [1, 4], [128, 16]])
                dst.offset = dst.offset + llo * 4
                nc.vector.tensor_copy(dst, src)

            wend_after = {67 + 4 * llo: llo for llo in range(16)}
            wend_after.update({195 + 4 * (llo - 16): llo for llo in range(16, 32)})
            hb = None
            for lp in range(0, NL, 2):   # two lanes per PSUM tile: fewer,
                pp = psT.tile([128, 352], f32, name="pp")  # wider evicts;
                for di in range(2):      # 0:64 transposes, 64:320 h rows
                    lane = lp + di
                    lgp = (lane >> 6) & 1
                    ll = (lane >> 7) * 64 + (lane & 63)
                    zsl = zPr[32 * lgp:32 * lgp + 32, :].copy()
                    zsl.ap = bass_rust.VecI64Pair([[zsl.ap[0][0], 32], [128, 128]])
                    zsl.offset = zsl.offset + 128 + ll
                    nc.tensor.matmul(pp[:, di * 32:di * 32 + 32], zsl,
                                     id32[32 * lgp:32 * lgp + 32, :],
                                     is_transpose=True, start=True, stop=True,
                                     skip_group_check=True)
                    nc.tensor.matmul(pp[0:1, 64 + di * 128:192 + di * 128],
                                     chs[32 * lgp:32 * lgp + 32, ll:ll + 1],
                                     zsl, start=True, stop=True, skip_group_check=True)
                b = lp >> 7
                f = lp & 127
                nc.scalar.copy(pts[:, lp * 32:(lp + 2) * 32], pp[:, 0:64])
                if f % 4 == 0:
                    hb = hbp.tile([1, 4 * 128], f32, name="hb")
                nc.vector.tensor_copy(hb[:, (f % 4) * 128:(f % 4) * 128 + 256],
                                      pp[0:1, 64:320])
                if f % 4 == 2:
                    # h[0] += Dt in f32 BEFORE the bf16 narrowing: on heavily
                    # attenuated lanes Dt cancels h[0] and bf16-then-add breaks
                    # the cancellation (h[0] sits reversed at chunk col k*128+127)
                    hDv = hb[:, :].copy()
                    hDv.ap = bass_rust.VecI64Pair([[hDv.ap[0][0], 1], [128, 4]])
                    hDv.offset = hDv.offset + 127
                    nc.gpsimd.tensor_tensor(hDv, hDv, Dts[0:1, lp - 2:lp + 2],
                                            op=Alu.add)
                    hb2 = hbp.tile([1, 4 * 128], bf16, name="hb2")
                    nc.gpsimd.tensor_copy(hb2[:], hb[:])
                    src = hb2[:, :].copy()
                    src.ap = bass_rust.VecI64Pair([[src.ap[0][0], 1], [128, 4], [1, 128]])
                    nc.sync.dma_start(
                        hzs[f - 2:f + 2, b * 264 + 9:b * 264 + 137], src)
                for lane in (lp, lp + 1):
                    if lane in wend_after:
                        emit_wend(wend_after[lane])

            # ---- z-scan (16 steps) + beta fold.  The swapped state zsw_j =
            # swap(z_j) obeys the SAME recurrence with swap(aBe) and swapped w
            # (aA is partner-symmetric), so both trajectories run as pure DVE
            # recurrences with NO per-step swap DMAs:
            #   z'   = aA.z   + aBe.zsw + w
            #   zsw' = aA.zsw + aBe2.z  + wsw,   aBe2 = swap(aBes)
            wsw = zsc.tile([64, 16 * 128], f32, name="wsw")
            aBe2 = zsc.tile([64, 128], f32, name="aBe2")
            for lgp in range(2):
                nc.sync.dma_start(wsw[32 * lgp:32 * lgp + 16, :], wendb[32 * lgp + 16:32 * lgp + 32, :])
                nc.sync.dma_start(wsw[32 * lgp + 16:32 * lgp + 32, :], wendb[32 * lgp:32 * lgp + 16, :])
                nc.sync.dma_start(aBe2[32 * lgp:32 * lgp + 16, :], aBes[32 * lgp + 16:32 * lgp + 32, :])
                nc.sync.dma_start(aBe2[32 * lgp + 16:32 * lgp + 32, :], aBes[32 * lgp:32 * lgp + 16, :])
            zbuf = zsc.tile([64, 16 * 128], f32, name="zbuf")
            zs2 = zsc.tile([64, 16 * 128], f32, name="zs2")
            t1 = zsc.tile([64, 128], f32, name="t1")
            t2 = zsc.tile([64, 128], f32, name="t2")
            u1 = zsc.tile([64, 128], f32, name="u1")
            u2 = zsc.tile([64, 128], f32, name="u2")
            nc.vector.memset(zbuf[:, 0:128], 0.0)
            nc.vector.memset(zs2[:, 0:128], 0.0)
            for j in range(1, 16):
                zprev = zbuf[:, (j - 1) * 128:j * 128]
                sprev = zs2[:, (j - 1) * 128:j * 128]
                wprev = wendb[:, (j - 1) * 128:j * 128]
                wsprev = wsw[:, (j - 1) * 128:j * 128]
                # z-chain on DVE, zsw-chain on Pool — independent per step
                nc.vector.tensor_tensor(t1[:], aAs[:], zprev, op=Alu.mult)
                nc.vector.tensor_tensor(t2[:], aBes[:], sprev, op=Alu.mult)
                nc.vector.tensor_tensor(t2[:], t1[:], t2[:], op=Alu.add)
                nc.vector.tensor_tensor(zbuf[:, j * 128:(j + 1) * 128], t2[:], wprev, op=Alu.add)
                nc.gpsimd.tensor_tensor(u1[:], aAs[:], sprev, op=Alu.mult)
                nc.gpsimd.tensor_tensor(u2[:], aBe2[:], zprev, op=Alu.mult)
                nc.gpsimd.tensor_tensor(u2[:], u1[:], u2[:], op=Alu.add)
                nc.gpsimd.tensor_tensor(zs2[:, j * 128:(j + 1) * 128], u2[:], wsprev, op=Alu.add)
            # beta in place: zbuf *= cA_t, zs2 *= cB_t, zbuf += zs2
            cab = cAs[:].unsqueeze(1).broadcast_to([64, 16, 128])
            cbb = cBs[:].unsqueeze(1).broadcast_to([64, 16, 128])
            z3 = zbuf[:].rearrange("p (j l) -> p j l", j=16)
            zs3 = zs2[:].rearrange("p (j l) -> p j l", j=16)
            nc.vector.tensor_tensor(z3, cab, z3, op=Alu.mult)
            nc.vector.tensor_tensor(zs3, cbb, zs3, op=Alu.mult)
            nc.vector.tensor_tensor(zbuf[:], zbuf[:], zs2[:], op=Alu.add)
            beta = zbuf

            # ---- per-lane A1 + B matmuls, evict, DMA out
            for grp in range(NL // 4):                 # 4 lanes per [64,512] psum
                pa = psA.tile([64, 256], f32, name="pa")
                for sl in range(4):
                    lane = grp * 4 + sl
                    b = lane >> 7
                    f = lane & 127
                    lgp = (lane >> 6) & 1
                    ll = (lane >> 7) * 64 + (lane & 63)
                    po = 32 * (sl & 1)
                    fo = 128 * (sl >> 1)
                    hd = hdp.tile([128, 128], bf16, name="hd")
                    # reversed-hdiag gather: hd[c, rp] = h[(127-rp)-c], all strides +1
                    # (issued on the Activation HWDGE queue to run parallel to
                    # the SP queue's other traffic)
                    src = hzs[f:f + 1, :].copy()
                    src.ap = bass_rust.VecI64Pair([[src.ap[0][0], 1], [1, 128], [1, 128]])
                    src.offset = src.offset + b * 264 + 9
                    # 3-way queue split: SP + Act HWDGE + Pool SWDGE
                    (nc.scalar, nc.sync, nc.gpsimd)[lane % 3].dma_start(hd[:], src)
                    nc.tensor.matmul(pa[po:po + 16, fo:fo + 128],
                                     X[:, lane * 16:lane * 16 + 16],
                                     hd[:, 127::-1], start=True, stop=False, skip_group_check=True)
                    # moving operand: S[r+1] at zPr col (127-r)*128 + ll
                    zrev = zPr[32 * lgp:32 * lgp + 32, :].copy()
                    zrev.ap = bass_rust.VecI64Pair([[zrev.ap[0][0], 32], [-128, 128]])
                    zrev.offset = zrev.offset + 127 * 128 + ll
                    nc.tensor.matmul(pa[po:po + 16, fo:fo + 128],
                                     beta[32 * lgp:32 * lgp + 32, ll::128],
                                     zrev,
                                     start=False, stop=True, skip_group_check=True)
                ybb = ybp.tile([64, 256], bf16, name="ybb")
                nc.vector.tensor_copy(ybb[0:16, :], pa[0:16, :])   # psum -> bf16
                nc.vector.tensor_copy(ybb[32:48, :], pa[32:48, :])
                # DMA out: src rows po=32*(sl&1) cover frames f0+sl for
                # sl in {0,2} (po=0) / {1,3} (po=32); one DMA per row pair
                lane0 = grp * 4
                b0_ = lane0 >> 7
                f0 = lane0 & 127
                for po, dlt in ((0, 0), (32, 1)):
                    srcy = ybb[po:po + 16, :].copy()
                    srcy.ap = bass_rust.VecI64Pair([[srcy.ap[0][0], 16], [128, 2], [1, 128]])
                    dsty = y_d[b0_:b0_ + 1, :].copy()
                    dsty.ap = bass_rust.VecI64Pair([[128, 16], [2 * FRAME, 2], [1, 128]])
                    dsty.offset = b0_ * N + (f0 + dlt) * FRAME
                    eng = nc.sync if (grp + dlt) % 2 == 0 else nc.scalar
                    eng.dma_start(dsty, srcy)

    # walrus rejects >1 sync-wait per instruction on this toolchain
    if not split_waits:
        return nc
    import concourse.mybir as mb2
    fn = nc.m.functions[0]
    for bb in fn.blocks:
        insts = bb.instructions
        i = 0
        while i < len(insts):
            inst = insts[i]
            si = inst.sync_info
            if si is not None and si.on_wait and len(si.on_wait) > 1:
                waits = list(si.on_wait)
                extra, keep = waits[:-1], waits[-1:]
                new_nops = []
                for k, w in enumerate(extra):
                    nop = mb2.InstNoOp(name=f"{inst.name}_wsplit{k}", ins=[], outs=[])
                    nop.engine = inst.engine
                    nop.sync_info = mb2.SyncInfo(on_wait=[w], on_update=[])
                    new_nops.append(nop)
                si.on_wait = keep
                insts[i:i] = new_nops
                i += len(new_nops)
            i += 1
    return nc


def _make_runner(nc):
    """Build a cached jitted callable for the SPMD bass program.

    Replicates concourse.bass2jax.run_bass_via_pjrt but (a) reuses one jit
    cache entry across kernel() calls (run_bass_via_pjrt builds a fresh
    closure per call, re-tracing and re-lowering each time), and (b)
    materializes the donated ExternalOutput zero-buffers ON DEVICE inside
    the jitted function instead of shipping host zeros over the axon tunnel.
    """
    import jax
    import jax.numpy as jnp
    from jax.sharding import Mesh, PartitionSpec
    from jax.experimental.shard_map import shard_map
    from concourse import mybir
    from concourse.bass2jax import (
        _bass_exec_p, install_neuronx_cc_hook, partition_id_tensor)

    install_neuronx_cc_hook()
    partition_name = nc.partition_id_tensor.name if nc.partition_id_tensor else None
    in_names, out_names, out_avals = [], [], []
    for alloc in nc.m.functions[0].allocations:
        if not isinstance(alloc, mybir.MemoryLocationSet):
            continue
        name = alloc.memorylocations[0].name
        if alloc.kind == "ExternalInput":
            if name != partition_name:
                in_names.append(name)
        elif alloc.kind == "ExternalOutput":
            out_names.append(name)
            out_avals.append(jax.core.ShapedArray(
                tuple(alloc.tensor_shape), mybir.dt.np(alloc.dtype)))
    n_params = len(in_names)
    all_names = in_names + out_names + ([partition_name] if partition_name else [])

    def _body(*args):
        operands = list(args)
        if partition_name:
            operands.append(partition_id_tensor())
        return tuple(_bass_exec_p.bind(
            *operands, out_avals=tuple(out_avals), in_names=tuple(all_names),
            out_names=tuple(out_names), lowering_input_output_aliases=(),
            sim_require_finite=True, sim_require_nnan=True, nc=nc))

    devices = jax.devices()[:N_CORES]
    mesh = Mesh(np.asarray(devices), ("core",))
    n_outs = len(out_names)
    inner = shard_map(
        _body, mesh=mesh,
        in_specs=(PartitionSpec("core"),) * (n_params + n_outs),
        out_specs=(PartitionSpec("core"),) * n_outs, check_rep=False)
    fn = jax.jit(inner, donate_argnums=tuple(range(n_params, n_params + n_outs)),
                 keep_unused=True)

    # Zero output buffers are made ON DEVICE (they're donated into fn, so a
    # fresh set is needed every call — but never shipped over the tunnel).
    from jax.sharding import NamedSharding
    shard = NamedSharding(mesh, PartitionSpec("core"))
    zeros_fn = jax.jit(
        lambda: tuple(jnp.zeros((N_CORES * a.shape[0], *a.shape[1:]), a.dtype)
                      for a in out_avals),
        out_shardings=tuple(shard for _ in out_avals))

    return fn, zeros_fn, in_names, out_names, out_avals


def kernel(audio, params):
    import jax
    from jax.sharding import Mesh, PartitionSpec, NamedSharding
    from concourse import mybir
    BF16 = mybir.dt.np(mybir.dt.bfloat16)
    audio = np.asarray(audio, dtype=np.float32)
    params = np.asarray(params, dtype=np.float32)
    if "fn" not in _prog_cache:
        nc = _build_program()
        _prog_cache["nc"] = nc
        _prog_cache["fn"] = _make_runner(nc)
        mesh = Mesh(np.asarray(jax.devices()[:N_CORES]), ("core",))
        _prog_cache["shard"] = NamedSharding(mesh, PartitionSpec("core"))
    fn, zeros_fn, in_names, out_names, out_avals = _prog_cache["fn"]
    # cheap bf16 cast, then start the 8 MB upload NOW; it overlaps with the
    # parameter-table computation below
    x_dev = jax.device_put(audio.astype(BF16), _prog_cache["shard"])
    tables = host_tables_all(audio, params)
    tables["xT"] = x_dev
    concat_in = [tables[nm] for nm in in_names]
    zs = _prog_cache.pop("zs", None)
    if zs is None:
        zs = zeros_fn()
    out_arrs = fn(*concat_in, *zs)
    # prefetch donated zero buffers for the NEXT call; overlaps with fetch
    _prog_cache["zs"] = zeros_fn()
    yi = out_names.index("y")
    out = np.asarray(out_arrs[yi]).reshape(N_CORES * BPC, N)
    return out.astype(np.float32)


if __name__ == "__main__":
    rng = np.random.default_rng(0)
    a = rng.standard_normal((B_FULL, N)).astype(np.float32)
    p = rng.random((B_FULL, 50, F)).astype(np.float32)
    y = kernel(a, p)
    print(y.shape, np.abs(y).max())

